# revision 1
# baseline (speedup 1.0000x reference)
"""Trainium2 Bass kernel for the EpisodicMemoryModule layer.

Problem (hardcoded shapes): B=64, F=256, E=U=512, MEMORY_HOPS=2.
Sharding: data-parallel over batch -> 8 cores x 8 rows each; weights replicated.

Per-core algorithm (all layouts "transposed": feature dim on partitions):
  P1:   X^T = (facts @ k_ep + b_ep0 + b_ep1)^T           stored fp16 [128, 12, Bc, F]
  hop loop:
    gate: feat^T chunks (fp16, built on the fly) @ W1 -> tanh -> @ W2 -> sigmoid
          sigmoid/gate broadcast to 128 partitions via a K=1 ones-matmul.
    scan: 256 sequential GRU steps; recurrent matmul h @ rk_ep via 48
          LDW+MM pairs per step (stationary fp16 rk tiles [128,128], moving
          fp16 h^T [128,8]); elementwise fused with scalar_tensor_tensor and
          tanh-only activations (sigmoid(x) = 0.5 + 0.5 tanh(x/2)).
    mem GRU + output store.

Matmuls run in fp16 (fp32 PSUM accumulation); GRU state stays fp32.
Measured emulation error vs fp32 reference: ~8e-4 relative.
"""

import numpy as np

B, F_FULL, E, U, HOPS, MESH = 64, 256, 512, 512, 2, 8
Bc = B // MESH
P = 128
CE = E // P          # 4  K-chunks of the 512 feature dim
NJ = 3 * U // P      # 12 output chunks of the 3U GRU matmul dim

# wsb segment bases, in units of 128-column tiles
SEG_RK = 0            # rk_ep   48 tiles, idx j*CE+c
SEG_KEP = 48          # k_ep    48 tiles
SEG_W1 = 96           # W1      64 tiles, idx kk*4+n
SEG_W1H0 = 160        # W1 folded for hop0 (m == questions == 0.1), 32 tiles
SEG_KM = 192          # k_mem   48 tiles
SEG_RKM = 240         # rk_mem  48 tiles
NTILES = 288
# extra columns after the tiles: W2T [128,4] fp16, then bias rows (row 0 only)
COL_W2 = NTILES * P                 # 4 cols
COL_BMZR = COL_W2 + 4               # 12*128 cols  (bias_mem0+bias_mem1, zr part)
COL_BMH0 = COL_BMZR + 8 * P         # 4*128 cols   (bias_mem0, h part)
COL_BMH1 = COL_BMH0 + 4 * P         # 4*128 cols   (bias_mem1, h part)
NW = COL_BMH1 + 4 * P


def _st_tiles(w, nk, nm):
    """[nk*128, nm*128] -> [128, nm*nk, 128] with tile idx = m*nk + k."""
    t = w.reshape(nk, P, nm, P)
    return t.transpose(1, 2, 0, 3).reshape(P, nm * nk, P)


def _w1_tiles(w, nk):
    """[nk*128, 512] -> [128, nk*4, 128] with tile idx = kk*4 + n."""
    t = w.reshape(nk, P, 4, P)
    return t.transpose(1, 0, 2, 3).reshape(P, nk * 4, P)


def build_host_weights(W1, b1, W2, b2, k_ep, rk_ep, bias_ep, k_mem, rk_mem, bias_mem):
    """Pack all weights into one [128, NW] fp16 array + the fp32 ep-bias vector."""
    wsb = np.zeros((P, NW), np.float16)
    wsb[:, SEG_RK * P:SEG_KEP * P] = _st_tiles(rk_ep, CE, NJ).reshape(P, -1)
    wsb[:, SEG_KEP * P:SEG_W1 * P] = _st_tiles(k_ep, CE, NJ).reshape(P, -1)
    wsb[:, SEG_W1 * P:SEG_W1H0 * P] = _w1_tiles(W1, 16).reshape(P, -1)
    w1h0 = np.concatenate([W1[:512] + W1[512:1024], W1[1024:1536] + W1[1536:]], 0)
    wsb[:, SEG_W1H0 * P:SEG_KM * P] = _w1_tiles(w1h0, 8).reshape(P, -1)
    wsb[:, SEG_KM * P:SEG_RKM * P] = _st_tiles(k_mem, CE, NJ).reshape(P, -1)
    wsb[:, SEG_RKM * P:NTILES * P] = _st_tiles(rk_mem, CE, NJ).reshape(P, -1)
    wsb[:, COL_W2:COL_W2 + 4] = W2.reshape(4, P).T
    # mem bias rows (row 0). row-major over the 3U dim.
    bm_sum = (bias_mem[0] + bias_mem[1]).astype(np.float16)
    wsb[0, COL_BMZR:COL_BMH0] = bm_sum[:8 * P]
    wsb[0, COL_BMH0:COL_BMH1] = bias_mem[0][8 * P:].astype(np.float16)
    wsb[0, COL_BMH1:NW] = bias_mem[1][8 * P:].astype(np.float16)
    # ep bias (fp32, added on the X psum->sbuf copy, per-partition scalar)
    bsum = (bias_ep[0] + bias_ep[1]).astype(np.float32)          # [1536]
    bsumT = bsum.reshape(NJ, P).T.copy()                         # [128, 12]
    # gate b1 folded into the psum->Y tanh step needs per-partition too
    b1T = np.asarray(b1, np.float32).reshape(4, P).T.copy()      # [128, 4]
    return wsb, bsumT, b1T, float(np.asarray(b2).reshape(-1)[0])


def build_nc(F):
    import concourse.bass as bass
    import concourse.mybir as mybir
    import concourse.tile as tile
    from concourse import bacc
    from concourse.alu_op_type import AluOpType as alu

    F16, F32, U16 = mybir.dt.float16, mybir.dt.float32, mybir.dt.uint16
    TANH = mybir.ActivationFunctionType.Tanh
    NCOL = Bc * F
    W = min(512, NCOL)
    assert NCOL % W == 0
    NCC = NCOL // W
    NB = W // F if W >= F else 0
    assert NB * F == W

    nc = bacc.Bacc("TRN2", target_bir_lowering=False)
    facts_d = nc.declare_dram_parameter("factsT16", [E, Bc, F], F16, isOutput=False)
    wsb_d = nc.declare_dram_parameter("wsb", [P, NW], F16, isOutput=False)
    bsum_d = nc.declare_dram_parameter("bsumT", [NJ * P], F32, isOutput=False)
    b1_d = nc.declare_dram_parameter("b1T", [4 * P], F32, isOutput=False)
    b2_d = nc.declare_dram_parameter("b2v", [1, 1], F32, isOutput=False)
    # stored u-major per core: out[hop, c, p, b] = memory[b, 128c+p]; host transposes
    out_d = nc.declare_dram_parameter("out", [HOPS, CE, P, Bc], F32, isOutput=True)

    with tile.TileContext(nc) as tc:
        with tc.tile_pool(name="persist", bufs=1) as PS:
            wsb = PS.tile([P, NW], F16)
            nc.sync.dma_start(out=wsb[:], in_=wsb_d[:, :])

            def wt(seg, idx):
                return wsb[:, (seg + idx) * P:(seg + idx + 1) * P]

            fT = PS.tile([P, CE, Bc, F], F16)
            nc.sync.dma_start(out=fT[:], in_=facts_d.rearrange("(c p) b f -> p c b f", p=P))
            bsumT = PS.tile([P, NJ], F32)
            nc.sync.dma_start(out=bsumT[:], in_=bsum_d.rearrange("(j p) -> p j", p=P))
            b1T = PS.tile([P, 4], F32)
            nc.sync.dma_start(out=b1T[:], in_=b1_d.rearrange("(n p) -> p n", p=P))
            b2v = PS.tile([1, 1], F32)
            nc.sync.dma_start(out=b2v[:], in_=b2_d[:, :])

            X16 = PS.tile([P, NJ, Bc, F], F16)
            Grep = PS.tile([P, NCOL], F32)
            Y = PS.tile([P, 4, NCOL], F16)
            g16 = PS.tile([1, NCOL], F16)
            h = PS.tile([P, CE, Bc], F32)
            hq = PS.tile([P, CE, Bc], F16)
            mT = PS.tile([P, CE, Bc], F32)
            mq = PS.tile([P, CE, Bc], F16)
            ones_r = PS.tile([1, P], F16)
            ones_b = PS.tile([1, Bc], F16)
            nc.vector.memset(ones_r[:], 1.0)
            nc.vector.memset(ones_b[:], 1.0)
            nc.vector.memset(mT[:], 0.1)
            nc.vector.memset(mq[:], 0.1)

            # ---------------- P1: X = facts @ k_ep + bsum ----------------
            with tc.tile_pool(name="xps", bufs=2, space="PSUM") as XPS:
                for jj in range(NJ):
                    xp = XPS.tile([P, NCOL], F32)
                    for c in range(CE):
                        for cc in range(NCC):
                            nc.tensor.matmul(
                                xp[:, cc * W:(cc + 1) * W],
                                wt(SEG_KEP, jj * CE + c),
                                fT[:, c].rearrange("p b f -> p (b f)")[:, cc * W:(cc + 1) * W],
                                start=(c == 0), stop=(c == CE - 1))
                    nc.vector.tensor_scalar(
                        X16[:, jj].rearrange("p b f -> p (b f)"), xp[:],
                        bsumT[:, jj:jj + 1], None, alu.add)

            # ---------------- hop loop ----------------
            for hop in range(HOPS):
                KK = 8 if hop == 0 else 16
                seg_w1 = SEG_W1H0 if hop == 0 else SEG_W1

                # ---- gate ----
                with tc.tile_pool(name="fpps", bufs=1, space="PSUM") as FPS, \
                     tc.tile_pool(name="gpps", bufs=2, space="PSUM") as GPS, \
                     tc.tile_pool(name="bpps", bufs=2, space="PSUM") as BPS, \
                     tc.tile_pool(name="featp", bufs=3) as FP, \
                     tc.tile_pool(name="gtmp", bufs=2) as GT:
                    for cc in range(NCC):
                        ccs = slice(cc * W, (cc + 1) * W)
                        fp = FPS.tile([P, 4, W], F32)
                        for kk in range(KK):
                            blk, c = kk // CE, kk % CE
                            if hop == 0:
                                blk *= 2  # folded: 0 -> 0.1*facts, 1 -> |facts-0.1|
                            src = fT[:, c].rearrange("p b f -> p (b f)")[:, ccs]
                            featc = FP.tile([P, W], F16, tag="featc")
                            if blk == 0:
                                nc.vector.tensor_scalar(featc[:], src, 0.1, None, alu.mult)
                            elif blk == 2:
                                # |x - 0.1| via fp16 sign-bit mask (abs_max not in ISA)
                                nc.vector.tensor_scalar(featc[:], src, 0.1, None, alu.subtract)
                                nc.vector.tensor_scalar(featc[:].bitcast(U16), featc[:].bitcast(U16),
                                                        0x7FFF, None, alu.bitwise_and)
                            else:
                                mb = mT[:, c, cc * NB:(cc + 1) * NB, None].broadcast_to([P, NB, F])
                                s3 = src.rearrange("p (b f) -> p b f", f=F)
                                f3 = featc[:].rearrange("p (b f) -> p b f", f=F)
                                if blk == 1:
                                    nc.vector.tensor_mul(f3, s3, mb)
                                else:
                                    nc.vector.tensor_sub(f3, s3, mb)
                                    nc.vector.tensor_scalar(featc[:].bitcast(U16), featc[:].bitcast(U16),
                                                            0x7FFF, None, alu.bitwise_and)
                            for n in range(4):
                                # psum groups are per 2KB bank: when W==512 each n
                                # slice is its own bank, else the tile is one bank.
                                if W == 512:
                                    st, sp = (kk == 0), (kk == KK - 1)
                                else:
                                    st = (kk == 0 and n == 0)
                                    sp = (kk == KK - 1 and n == 3)
                                nc.tensor.matmul(fp[:, n], wt(seg_w1, kk * 4 + n), featc[:],
                                                 start=st, stop=sp)
                        gp = GPS.tile([1, W], F32)
                        for n in range(4):
                            # Y = tanh(feat@W1 + b1)
                            nc.scalar.activation(Y[:, n, ccs], fp[:, n], TANH,
                                                 bias=b1T[:, n:n + 1])
                            nc.tensor.matmul(gp[:], wsb[:, COL_W2 + n:COL_W2 + n + 1],
                                             Y[:, n, ccs], start=(n == 0), stop=(n == 3))
                        # sigmoid(x+b2) = .5 + .5 tanh(.5x + .5*b2); b2v holds .5*b2
                        nc.scalar.activation(g16[:, ccs], gp[:], TANH, scale=0.5,
                                             bias=b2v[:, 0:1])
                        bp = BPS.tile([P, W], F32)
                        nc.tensor.matmul(bp[:], ones_r[:], g16[:, ccs], start=True, stop=True)
                        # Grep = 0.5*g = 0.25*tanh + 0.25
                        nc.vector.tensor_scalar(Grep[:, ccs], bp[:], 0.25, 0.25,
                                                alu.mult, alu.add)

                # ---- episode scan ----
                nc.vector.memset(h[:], 0.0)
                G3 = Grep[:].rearrange("p (b f) -> p b f", f=F)
                with tc.tile_pool(name="zrps", bufs=2, space="PSUM") as ZPS, \
                     tc.tile_pool(name="hhps", bufs=2, space="PSUM") as HPS, \
                     tc.tile_pool(name="sew", bufs=3) as SE:
                    for t in range(F):
                        grep_t = G3[:, None, :, t].broadcast_to([P, CE, Bc])
                        if t == 0:
                            t24 = SE.tile([P, 8, Bc], F32, tag="t24")
                            nc.scalar.activation(t24[:], X16[:, 0:8, :, 0], TANH, scale=0.5)
                            cand = SE.tile([P, CE, Bc], F32, tag="cand")
                            nc.scalar.activation(cand[:], X16[:, 8:12, :, 0], TANH)
                            na = SE.tile([P, CE, Bc], F32, tag="na")
                            nc.vector.scalar_tensor_tensor(
                                na[:], t24[:, 0:4], 1.0, grep_t, alu.subtract, alu.mult)
                            e = SE.tile([P, CE, Bc], F32, tag="e")
                            nc.vector.tensor_mul(e[:], na[:], cand[:])
                            nc.vector.tensor_scalar(hq[:], e[:], -1.0, None, alu.mult)
                            nc.vector.tensor_scalar(h[:], e[:], -1.0, None, alu.mult)
                            continue
                        zr = ZPS.tile([P, 8, Bc], F32, tag="zr")
                        hh = HPS.tile([P, 4, Bc], F32, tag="hh")
                        for j in range(8):
                            for c in range(CE):
                                nc.tensor.matmul(zr[:, j], wt(SEG_RK, j * CE + c), hq[:, c],
                                                 start=(j == 0 and c == 0),
                                                 stop=(j == 7 and c == CE - 1))
                        for j in range(8, 12):
                            for c in range(CE):
                                nc.tensor.matmul(hh[:, j - 8], wt(SEG_RK, j * CE + c), hq[:, c],
                                                 start=(j == 8 and c == 0),
                                                 stop=(j == 11 and c == CE - 1))
                        t13 = SE.tile([P, 8, Bc], F32, tag="t13")
                        nc.vector.tensor_add(t13[:], X16[:, 0:8, :, t], zr[:])
                        t24 = SE.tile([P, 8, Bc], F32, tag="t24")
                        nc.scalar.activation(t24[:], t13[:], TANH, scale=0.5)
                        u = SE.tile([P, CE, Bc], F32, tag="u")
                        nc.vector.scalar_tensor_tensor(
                            u[:], t24[:, 4:8], 1.0, hh[:], alu.add, alu.mult)
                        v = SE.tile([P, CE, Bc], F32, tag="v")
                        nc.vector.scalar_tensor_tensor(
                            v[:], u[:], 0.5, X16[:, 8:12, :, t], alu.mult, alu.add)
                        cand = SE.tile([P, CE, Bc], F32, tag="cand")
                        nc.scalar.activation(cand[:], v[:], TANH)
                        # h_new = h - na*(cand-h) = (na+1)*h - na*cand
                        #   ta = (na+1)*h depends only on (na, h): runs before cand
                        #   tb = na*cand is the only op serialized after cand
                        na = SE.tile([P, CE, Bc], F32, tag="na")
                        nc.vector.scalar_tensor_tensor(
                            na[:], t24[:, 0:4], 1.0, grep_t, alu.subtract, alu.mult)
                        ta = SE.tile([P, CE, Bc], F32, tag="ta")
                        nc.vector.scalar_tensor_tensor(
                            ta[:], na[:], 1.0, h[:], alu.add, alu.mult)
                        tb = SE.tile([P, CE, Bc], F32, tag="tb")
                        nc.vector.tensor_mul(tb[:], na[:], cand[:])
                        nc.vector.tensor_sub(hq[:], ta[:], tb[:])
                        nc.vector.tensor_sub(h[:], ta[:], tb[:])

                # ---- memory GRU ----
                with tc.tile_pool(name="mps", bufs=1, space="PSUM") as MPS, \
                     tc.tile_pool(name="mew", bufs=1) as ME:
                    eq = ME.tile([P, CE, Bc], F16, tag="eq")
                    nc.vector.tensor_copy(eq[:], h[:])
                    A = MPS.tile([P, 8, Bc], F32, tag="A")
                    Bp = MPS.tile([P, 4, Bc], F32, tag="Bp")
                    Cp = MPS.tile([P, 4, Bc], F32, tag="Cp")
                    for j in range(8):
                        for c in range(CE):
                            nc.tensor.matmul(A[:, j], wt(SEG_KM, j * CE + c), eq[:, c],
                                             start=(j == 0 and c == 0), stop=False)
                            nc.tensor.matmul(A[:, j], wt(SEG_RKM, j * CE + c), mq[:, c],
                                             start=False, stop=False)
                        nc.tensor.matmul(A[:, j], wsb[0:1, COL_BMZR + j * P:COL_BMZR + (j + 1) * P],
                                         ones_b[:], start=False, stop=(j == 7))
                    for j in range(4):
                        for c in range(CE):
                            nc.tensor.matmul(Cp[:, j], wt(SEG_KM, (j + 8) * CE + c), eq[:, c],
                                             start=(j == 0 and c == 0), stop=False)
                        nc.tensor.matmul(Cp[:, j], wsb[0:1, COL_BMH0 + j * P:COL_BMH0 + (j + 1) * P],
                                         ones_b[:], start=False, stop=(j == 3))
                    for j in range(4):
                        for c in range(CE):
                            nc.tensor.matmul(Bp[:, j], wt(SEG_RKM, (j + 8) * CE + c), mq[:, c],
                                             start=(j == 0 and c == 0), stop=False)
                        nc.tensor.matmul(Bp[:, j], wsb[0:1, COL_BMH1 + j * P:COL_BMH1 + (j + 1) * P],
                                         ones_b[:], start=False, stop=(j == 3))
                    t24m = ME.tile([P, 8, Bc], F32, tag="t24m")
                    nc.scalar.activation(t24m[:], A[:], TANH, scale=0.5)
                    um = ME.tile([P, CE, Bc], F32, tag="um")
                    nc.vector.scalar_tensor_tensor(
                        um[:], t24m[:, 4:8], 1.0, Bp[:], alu.add, alu.mult)
                    vm = ME.tile([P, CE, Bc], F32, tag="vm")
                    nc.vector.scalar_tensor_tensor(
                        vm[:], um[:], 0.5, Cp[:], alu.mult, alu.add)
                    candm = ME.tile([P, CE, Bc], F32, tag="candm")
                    nc.scalar.activation(candm[:], vm[:], TANH)
                    nzm = ME.tile([P, CE, Bc], F32, tag="nzm")
                    nc.vector.tensor_scalar(nzm[:], t24m[:, 0:4], 1.0, 0.5,
                                            alu.subtract, alu.mult)
                    dm = ME.tile([P, CE, Bc], F32, tag="dm")
                    nc.vector.tensor_sub(dm[:], candm[:], mT[:])
                    em = ME.tile([P, CE, Bc], F32, tag="em")
                    nc.vector.tensor_mul(em[:], nzm[:], dm[:])
                    nc.vector.tensor_sub(mq[:], mT[:], em[:])
                    nc.vector.tensor_sub(mT[:], mT[:], em[:])
                nc.sync.dma_start(out=out_d[hop].rearrange("c p b -> p c b"),
                                  in_=mT[:, :, :])
    nc.compile()
    return nc


_CACHE = {}


def _get_nc(F):
    if F not in _CACHE:
        _CACHE[F] = build_nc(F)
    return _CACHE[F]


def host_inputs(inputs, F=F_FULL):
    """Build per-core in_maps from the full-problem input dict."""
    facts = np.asarray(inputs["facts"], np.float32)[:, :F]
    wsb, bsumT, b1T, b2f = build_host_weights(
        np.asarray(inputs["W1"], np.float32), np.asarray(inputs["b1"], np.float32),
        np.asarray(inputs["W2"], np.float32), np.asarray(inputs["b2"], np.float32),
        np.asarray(inputs["k_ep"], np.float32), np.asarray(inputs["rk_ep"], np.float32),
        np.asarray(inputs["bias_ep"], np.float32), np.asarray(inputs["k_mem"], np.float32),
        np.asarray(inputs["rk_mem"], np.float32), np.asarray(inputs["bias_mem"], np.float32))
    bsum_flat = bsumT.T.reshape(-1).copy()   # [(j p)] order
    b1_flat = b1T.T.reshape(-1).copy()
    b2v = np.full((1, 1), 0.5 * b2f, np.float32)
    in_maps = []
    for i in range(MESH):
        sh = facts[i * Bc:(i + 1) * Bc]                       # [Bc, F, E]
        factsT16 = np.ascontiguousarray(sh.transpose(2, 0, 1)).astype(np.float16)
        in_maps.append({
            "factsT16": factsT16, "wsb": wsb, "bsumT": bsum_flat,
            "b1T": b1_flat, "b2v": b2v,
        })
    return in_maps


def unpack_out(o):
    """[HOPS, CE, P, Bc] device layout -> [HOPS, Bc, U]."""
    return np.ascontiguousarray(np.asarray(o).transpose(0, 3, 1, 2)).reshape(HOPS, Bc, U)


def run(inputs, trace=False, **kw):
    from concourse.bass_utils import run_bass_kernel_spmd
    nc = _get_nc(F_FULL)
    in_maps = host_inputs(inputs, F_FULL)
    res = run_bass_kernel_spmd(nc, in_maps, list(range(MESH)), trace=trace, **kw)
    outs = [unpack_out(res.results[i]["out"]) for i in range(MESH)]
    return np.concatenate(outs, axis=1).astype(np.float32), res


def kernel(**inputs):
    return run(inputs)[0]

